# revision 5
# baseline (speedup 1.0000x reference)
import functools

import jax
import jax.numpy as jnp
import numpy as np

# Problem constants (hardcoded per spec: nn_ActorSvgd, B=2048, N=32, D=6)
B = 2048
N = 32
D = 6
OBS = 17
H = 256
STEPS = 10
LR = 0.1
LIMIT = 1.0
LOG_NP1 = np.float32(np.log(N + 1.0))

M = 8  # number of NeuronCores; shard batch axis B across them
BL = B // M  # envs per core


def _q1(obs, X, W1, b1, W2, b2, W3, b3):
    h = jax.nn.relu(jnp.concatenate([obs, X], axis=-1) @ W1 + b1)
    h = jax.nn.relu(h @ W2 + b2)
    return (h @ W3 + b3).squeeze(-1)


def _median_low(ds, k):
    """Lower median via bisection + masked max: k-th smallest (1-indexed) of each
    row of ds (values >= 0). Avoids sort, which neuronx-cc cannot compile."""
    hi = jnp.max(ds, axis=1)
    lo = jnp.zeros_like(hi)
    for _ in range(26):
        mid = 0.5 * (lo + hi)
        c = jnp.sum((ds <= mid[:, None]).astype(jnp.float32), axis=1)
        ge = c >= k
        hi = jnp.where(ge, mid, hi)
        lo = jnp.where(ge, lo, mid)
    # largest element <= hi is the k-th smallest once the bracket is tight
    return jnp.max(jnp.where(ds <= hi[:, None], ds, -1.0), axis=1)


def _rbf(X, Bl):
    Xd = jax.lax.stop_gradient(X)
    diff = X[:, :, None, :] - Xd[:, None, :, :]
    dist_sq = jnp.sum(diff * diff, axis=-1)
    nn_ = N * N
    med = _median_low(
        jax.lax.stop_gradient(dist_sq).reshape(-1, nn_), (nn_ - 1) // 2 + 1
    )
    h = med / (2.0 * LOG_NP1)
    gamma = 1.0 / (1e-8 + 2.0 * h)
    gamma = gamma[:, None, None]
    kappa = jnp.exp(-gamma * dist_sq)
    kappa_grad = -2.0 * (diff * gamma[..., None]) * kappa[..., None]
    return kappa, diff, gamma, kappa_grad


def _svgd_step(obs, a, logp, W1, b1, W2, b2, W3, b3):
    """One SVGD step on one shard of BL envs (exact reference math)."""
    Bl = BL

    def score_fn(Xf):
        return jax.grad(lambda Xg: _q1(obs, Xg, W1, b1, W2, b2, W3, b3).sum())(Xf)

    score = score_fn(a)
    X = a.reshape(Bl, N, D)
    s = score.reshape(Bl, N, D)
    K_XX, K_diff, K_gamma, K_grad = _rbf(X, Bl)
    Kd = jax.lax.stop_gradient(K_XX)
    kd_s = (Kd[:, :, :, None] * s[:, None, :, :]).sum(axis=2)
    phi = (kd_s - K_grad.sum(axis=2)) / N
    line_4 = (K_grad * s[:, None, :, :]).sum(-1).mean(-1)
    line_5 = (
        -2.0 * K_gamma[:, :, 0] * ((-K_grad * K_diff).sum(-1) - D * K_XX).mean(-1)
    )
    logp = logp - LR * (line_4 + line_5)
    a = jnp.clip(a + LR * phi.reshape(Bl * N, D), -LIMIT, LIMIT)
    a = jax.lax.stop_gradient(a)
    return a, logp


_pmapped = None


def _get_pmapped():
    global _pmapped
    if _pmapped is None:
        _pmapped = jax.pmap(
            _svgd_step,
            in_axes=(0, 0, 0, None, None, None, None, None, None),
            out_axes=(0, 0),
        )
    return _pmapped


def kernel(obs, a, W1, b1, W2, b2, W3, b3):
    obs = np.asarray(obs, dtype=np.float32).reshape(M, BL * N, OBS)
    a_in = np.asarray(a, dtype=np.float32).reshape(M, BL * N, D)
    logp = np.zeros((M, BL, N), dtype=np.float32)
    W1 = np.asarray(W1, dtype=np.float32)
    b1 = np.asarray(b1, dtype=np.float32)
    W2 = np.asarray(W2, dtype=np.float32)
    b2 = np.asarray(b2, dtype=np.float32)
    W3 = np.asarray(W3, dtype=np.float32)
    b3 = np.asarray(b3, dtype=np.float32)

    fn = _get_pmapped()
    a_cur, logp_cur = a_in, logp
    for _ in range(STEPS):
        a_cur, logp_cur = fn(obs, a_cur, logp_cur, W1, b1, W2, b2, W3, b3)
    a_out = np.asarray(a_cur).reshape(B * N, D).astype(np.float32)
    logp_out = np.asarray(logp_cur).reshape(B, N).astype(np.float32)
    return a_out, logp_out


# revision 11
# speedup vs baseline: 1.1483x; 1.1483x over previous
import functools

import jax
import jax.numpy as jnp
import numpy as np

# Problem constants (hardcoded per spec: nn_ActorSvgd, B=2048, N=32, D=6)
B = 2048
N = 32
D = 6
OBS = 17
H = 256
STEPS = 10
LR = 0.1
LIMIT = 1.0
LOG_NP1 = np.float32(np.log(N + 1.0))

M = 8  # number of NeuronCores; shard batch axis B across them
BL = B // M  # envs per core


def _q1(obs, X, W1, b1, W2, b2, W3, b3):
    h = jax.nn.relu(jnp.concatenate([obs, X], axis=-1) @ W1 + b1)
    h = jax.nn.relu(h @ W2 + b2)
    return (h @ W3 + b3).squeeze(-1)


def _median_low(ds, k):
    """Lower median via bisection + masked max: k-th smallest (1-indexed) of each
    row of ds (values >= 0). Avoids sort, which neuronx-cc cannot compile."""
    hi = jnp.max(ds, axis=1)
    lo = jnp.zeros_like(hi)
    for _ in range(26):
        mid = 0.5 * (lo + hi)
        c = jnp.sum((ds <= mid[:, None]).astype(jnp.float32), axis=1)
        ge = c >= k
        hi = jnp.where(ge, mid, hi)
        lo = jnp.where(ge, lo, mid)
    # largest element <= hi is the k-th smallest once the bracket is tight
    return jnp.max(jnp.where(ds <= hi[:, None], ds, -1.0), axis=1)


def _rbf(X, Bl):
    Xd = jax.lax.stop_gradient(X)
    diff = X[:, :, None, :] - Xd[:, None, :, :]
    dist_sq = jnp.sum(diff * diff, axis=-1)
    nn_ = N * N
    med = _median_low(
        jax.lax.stop_gradient(dist_sq).reshape(-1, nn_), (nn_ - 1) // 2 + 1
    )
    h = med / (2.0 * LOG_NP1)
    gamma = 1.0 / (1e-8 + 2.0 * h)
    gamma = gamma[:, None, None]
    kappa = jnp.exp(-gamma * dist_sq)
    kappa_grad = -2.0 * (diff * gamma[..., None]) * kappa[..., None]
    return kappa, diff, gamma, kappa_grad


STEPS_PER_CALL = 1


def _svgd_step(obs, a, logp, W1, b1, W2, b2, W3, b3):
    """One SVGD step on one shard of BL envs (exact reference math)."""
    Bl = BL

    def score_fn(Xf):
        return jax.grad(lambda Xg: _q1(obs, Xg, W1, b1, W2, b2, W3, b3).sum())(Xf)

    score = score_fn(a)
    X = a.reshape(Bl, N, D)
    s = score.reshape(Bl, N, D)
    K_XX, K_diff, K_gamma, K_grad = _rbf(X, Bl)
    Kd = jax.lax.stop_gradient(K_XX)
    kd_s = (Kd[:, :, :, None] * s[:, None, :, :]).sum(axis=2)
    phi = (kd_s - K_grad.sum(axis=2)) / N
    line_4 = (K_grad * s[:, None, :, :]).sum(-1).mean(-1)
    line_5 = (
        -2.0 * K_gamma[:, :, 0] * ((-K_grad * K_diff).sum(-1) - D * K_XX).mean(-1)
    )
    logp = logp - LR * (line_4 + line_5)
    a = jnp.clip(a + LR * phi.reshape(Bl * N, D), -LIMIT, LIMIT)
    a = jax.lax.stop_gradient(a)
    return a, logp


_pmapped = None


def _get_pmapped():
    global _pmapped
    if _pmapped is None:

        def _multi(obs, a, logp, W1, b1, W2, b2, W3, b3):
            for _ in range(STEPS_PER_CALL):
                a, logp = _svgd_step(obs, a, logp, W1, b1, W2, b2, W3, b3)
            return a, logp

        _pmapped = jax.pmap(
            _multi,
            in_axes=(0, 0, 0, None, None, None, None, None, None),
            out_axes=(0, 0),
        )
    return _pmapped


def kernel(obs, a, W1, b1, W2, b2, W3, b3):
    obs = np.asarray(obs, dtype=np.float32).reshape(M, BL * N, OBS)
    a_in = np.asarray(a, dtype=np.float32).reshape(M, BL * N, D)
    logp = np.zeros((M, BL, N), dtype=np.float32)
    W1 = np.asarray(W1, dtype=np.float32)
    b1 = np.asarray(b1, dtype=np.float32)
    W2 = np.asarray(W2, dtype=np.float32)
    b2 = np.asarray(b2, dtype=np.float32)
    W3 = np.asarray(W3, dtype=np.float32)
    b3 = np.asarray(b3, dtype=np.float32)

    fn = _get_pmapped()
    a_cur, logp_cur = a_in, logp
    for _ in range(STEPS // STEPS_PER_CALL):
        a_cur, logp_cur = fn(obs, a_cur, logp_cur, W1, b1, W2, b2, W3, b3)
    a_out = np.asarray(a_cur).reshape(B * N, D).astype(np.float32)
    logp_out = np.asarray(logp_cur).reshape(B, N).astype(np.float32)
    return a_out, logp_out
